# revision 48
# baseline (speedup 1.0000x reference)
"""Multi-head self-attention (B=16, N=1024, D=768, H=12) on 8 TRN2 NeuronCores.

Data-parallel over batch (2 batches per core, weights replicated, no
collectives). Per core, one fused Bass/Tile kernel. ~354us HW exec
(v1 baseline: 453.8us), rel err 3.7e-4.

Layout: token 8p+t lives at partition p, slot t (attention is
permutation-invariant over tokens; undone at the out DMA). QT/KT in
[dim, tok] f16; V_aug = [x W_v | ones-col per head]; per head
S^T = K Q^T (row-tiled pairs: the two K=64 matmuls of a head pair run
CONCURRENTLY in the top/bottom PE array halves, tile_position
auto-derived from base_partition), E = exp(S^T*scale) on ACT,
O^T = V_aug^T E with row 64 = softmax denominator (ones column is free:
matmul cost is N-streaming, M=65 vs 64 costs nothing). All matmul
operands f16 (fp32 runs 1/4 rate and breaks HAM warm-up), PSUM f32.

What this version does beyond the v1 structure (each item measured):
  - host-side f16 casts for x/W_qkv/W_proj, f16 output (halves DMA
    bytes; on-chip weight-cast DVE work removed)
  - DMA issue engines chosen deliberately: each dma_start costs ~0.6us
    of the ISSUING engine's time, so weights issue from the otherwise-
    idle gpsimd queue (ACT must stay free for transpose evacuation),
    x6 + qkv-bias ride sync; softmax-recip round-trip DMAs ride gpsimd
  - W_v columns land first, in their own tiles, so all 16 V groups run
    while the 2x larger Q/K columns are still streaming
  - x transposes as REGULAR matmuls (chunk^T @ I): ~56ns pipelined vs
    ~250ns transpose-mode, and they count as HAM activity; 4 share one
    PSUM bank -> single [128,512] evacuation copy (b0 on ACT, b1 DVE)
  - rank-1 bias matmuls removed: fused bias (W_proj^T b_v + b_proj)
    broadcast once to [128,768] f32, added by DVE during PSUM evacuation
  - attention m-steps batched in pairs: the two K=64 QK pair-slots sit
    adjacent (one PE tile-mode switch in/out per batch, ~+100ns each),
    sps double-buffered so QK(mb+1) never waits exp(mb)'s PSUM read
  - nh-outer pair order; fillers scheduled against pop deadlines:
    attention-0 hosts b0 Q/K groups + b1 transposes + b1 V, attention-1
    hosts b1 Q/K + b0 proj + half of b1 proj; pair (jp, nh) needs BOTH
    nh-halves of its K tile (keys span all tokens) before it starts
  - O^T + denominator evacuated from PSUM immediately (anything slower
    -- e.g. DMA round-trips inside the ot lifetime -- stalls the next
    pair's AV by 2-4us); recip must NOT read PSUM directly (HW gives
    wrong values; sim diverges); both heads share one recip/cast/mul
  - final pair's softmax broadcast via two rank-1 PE matmuls (PE is
    idle there) + keep-warm matmuls over the recip latency so the tail
    projections run at 8/8 clock
  - qk tiles share slots across batches (qk[1] first written only after
    attention-0 drains); at tiles are per-batch (sharing would deadlock
    the DVE FIFO against PE via the b0-proj fillers)

Engine budget at 354us: PE ~91% busy (the bottleneck), ACT ~60% (192
exps at ~1.1us), DVE ~55%, HBM ~8MB in / 3MB out per core.
"""

import numpy as np

_CACHE: dict = {}

P = 128
BL, N, D, H, HD = 2, 1024, 768, 12, 64
D3 = 3 * D
SCALE = float(HD) ** -0.5


def _build():
    import concourse.mybir as mybir
    import concourse.tile as tile
    from concourse import bacc
    from concourse.masks import make_identity

    dt = mybir.dt
    F32, F16 = dt.float32, dt.float16
    AF = mybir.ActivationFunctionType

    nc = bacc.Bacc("TRN2", target_bir_lowering=False, debug=False)
    x_d = nc.dram_tensor("x", [BL, N, D], F16, kind="ExternalInput").ap()
    wqkv_d = nc.dram_tensor("W_qkv", [D, D3], F16, kind="ExternalInput").ap()
    bqkv_d = nc.dram_tensor("b_qkv", [D3], F32, kind="ExternalInput").ap()
    wproj_d = nc.dram_tensor("W_proj", [D, D], F16, kind="ExternalInput").ap()
    bproj_d = nc.dram_tensor("b_proj", [D], F32, kind="ExternalInput").ap()
    out_d = nc.dram_tensor("out", [BL, N, D], F16, kind="ExternalOutput").ap()
    # token-interleaved views: partition p, slot t <-> token 8p+t
    x_il = x_d.rearrange("b (p i) d -> b p (i d)", p=P)       # [2, 128, 6144]
    out_il = out_d.rearrange("b (p i) d -> b i p d", p=P)     # [2, 8, 128, 768]

    with tile.TileContext(nc) as tc:
        with tc.tile_pool(name="sb", bufs=1) as sb, \
             tc.tile_pool(name="dp", bufs=1, space="DRAM") as dp, \
             tc.tile_pool(name="ps", bufs=2, space="PSUM") as ps:

            # ---------- constants ----------
            ident = sb.tile([P, P], F16, tag="ident", bufs=1, name="ident")
            make_identity(nc, ident[:])
            ones_h = sb.tile([P, P], F16, tag="ones_h", bufs=1, name="ones_h")
            nc.vector.memset(ones_h[:], 1.0)

            # ---------- input DMAs ----------
            # sync queue: ONLY x6[0] (fast path for the first transposes).
            # scalar queue: biases, W_qkv (in d order, consumed
            # incrementally by the first QKV groups), then x6[1]
            # (needed only mid-attention-0), then W_proj.
            # DMA issue engines matter: each dma_start costs ~0.6us of the
            # ISSUING engine's time. ACT (scalar) must stay free for the
            # transpose-evacuation copies, so weights issue from the idle
            # gpsimd queue; x6 + the tiny qkv-bias ride the sync queue.
            bstg = sb.tile([18, P], F32, tag="bstg", bufs=1, name="bstg")
            nc.sync.dma_start(bstg[:], bqkv_d.rearrange("(j p) -> j p", p=P))
            x6 = {}
            for b in range(BL):
                x6[b] = sb.tile([P, 8 * D], F16, tag="x6", bufs=2, name="x6")
                for q in range(4):
                    nc.sync.dma_start(x6[b][:, 2 * D * q:2 * D * (q + 1)],
                                      x_il[b][:, 2 * D * q:2 * D * (q + 1)])
            # W_v columns land FIRST (own tiles -- a shared tile would
            # serialize V groups behind the whole-tile DMA set) so all 16
            # V groups can run while the 2x larger Q/K columns stream in.
            wq_h, wv_h, wp_h = [], [], []
            for d in range(6):
                t = sb.tile([P, D], F16, tag=f"wv{d}", bufs=1, name=f"wv{d}")
                nc.gpsimd.dma_start(t[:], wqkv_d[P * d:P * (d + 1), 2 * D:D3])
                wv_h.append(t)
            for d in range(6):
                t = sb.tile([P, 2 * D], F16, tag=f"wqkv{d}", bufs=1,
                            name=f"wqkv{d}")
                nc.gpsimd.dma_start(t[:], wqkv_d[P * d:P * (d + 1), 0:2 * D])
                wq_h.append(t)
            bproj_row = sb.tile([1, D], F32, tag="bproj_row", bufs=1,
                                name="bproj_row")
            nc.gpsimd.dma_start(bproj_row[:], bproj_d.unsqueeze(0))
            for d in range(6):
                t = sb.tile([P, D], F16, tag=f"wproj{d}", bufs=1, name=f"wproj{d}")
                nc.gpsimd.dma_start(t[:], wproj_d[P * d:P * (d + 1), :])
                wp_h.append(t)

            warm_h = sb.tile([P, 512], F16, tag="warm", bufs=1, name="warm_h")
            nc.vector.memset(warm_h[:], 0.0)

            def keep_warm(n):
                # ~3.4us of dense matmuls flips HAM to 8/8 (16 x 215ns)
                for wi in range(n):
                    wps = ps.tile([P, 512], F32, tag="mm", bufs=2, name="wps")
                    nc.tensor.matmul(wps[:], ones_h[:, 0:P], warm_h[:],
                                     start=True, stop=True)

            # PE warm-up: bridges the DGE-ramp / x6-DMA wait and flips HAM
            keep_warm(18)

            # ---------- x transposes as regular matmuls (chunk^T @ I) ----
            # 4 transposes share one PSUM bank; one [128,512] copy out.
            # b0 copies on ACT (idle in prologue), b1 on DVE.
            xT = {b: [sb.tile([P, N], F16, tag=f"xT{b}_{j}", bufs=1,
                              name=f"xT{b}_{j}") for j in range(6)]
                  for b in range(BL)}

            def emit_transpose(b, tb, j):
                bank = ps.tile([P, 512], F32, tag="mm", bufs=2, name="tbank")
                for k in range(4):
                    t = 4 * tb + k
                    nc.tensor.matmul(
                        bank[:, P * k:P * (k + 1)],
                        x6[b][:, D * t + P * j:D * t + P * (j + 1)],
                        ident[:], start=True, stop=True)
                if b == 0:
                    nc.scalar.copy(xT[b][j][:, 512 * tb:512 * (tb + 1)],
                                   bank[:])
                else:
                    nc.vector.tensor_copy(
                        xT[b][j][:, 512 * tb:512 * (tb + 1)], bank[:])

            for tb in range(2):
                for j in range(6):
                    emit_transpose(0, tb, j)

            # ---------- qkv bias transpose (tiny) ----------
            bstg_h = sb.tile([18, P], F16, tag="bstg_h", bufs=1, name="bstg_h")
            nc.vector.tensor_copy(bstg_h[:], bstg[:])
            btp = ps.tile([P, 18], F16, tag="mm", bufs=2, name="btp")
            nc.tensor.transpose(btp[:], bstg_h[:], ident[0:18, 0:18])
            bqkvT = sb.tile([P, 18], F32, tag="bqkvT", bufs=1, name="bqkvT")
            nc.vector.tensor_copy(bqkvT[:], btp[:])
            bv_h = sb.tile([P, 6], F16, tag="bv_h", bufs=1, name="bv_h")
            nc.vector.tensor_copy(bv_h[:], btp[:, 12:18])

            # ---------- tiles ----------
            # qk tags shared across batches (qk[1] first written only after
            # attention-0 fully drains -> clean slot WAR). at tags are
            # per-batch: attention-1's normalization writes at[1] while
            # b0-proj fillers still read at[0] -> sharing would deadlock
            # the DVE FIFO against the PE FIFO.
            qk = {b: [sb.tile([P, N], F16, tag=f"qk_{j}", bufs=1,
                              name=f"qk{j}") for j in range(12)]
                  for b in range(BL)}
            v = {b: [sb.tile([P, 12 * 65], F16, tag=f"v{b % 2}_{t}", bufs=1,
                             name=f"v{t}") for t in range(8)]
                 for b in range(BL)}
            at = {b: [sb.tile([P, N], F16, tag=f"at{b}_{j}", bufs=1,
                              name=f"at{j}") for j in range(6)]
                  for b in range(BL)}
            bfin_bc = sb.tile([P, D], F32, tag="bfin_bc", bufs=1,
                              name="bfin_bc")

            def emit_qkv_group(b, j, nh):
                qps = ps.tile([P, 512], F32, tag="mm", bufs=2, name="qps")
                for d in range(6):
                    nc.tensor.matmul(qps[:], wq_h[d][:, P * j:P * (j + 1)],
                                     xT[b][d][:, 512 * nh:512 * (nh + 1)],
                                     start=(d == 0), stop=(d == 5))
                nc.vector.tensor_scalar_add(
                    qk[b][j][:, 512 * nh:512 * (nh + 1)], qps[:],
                    bqkvT[:, j:j + 1])

            def emit_v_group(b, t, ci):
                c0, cw = ((0, 512), (512, 256))[ci]
                v3 = v[b][t].rearrange("p (h c) -> p h c", c=65)
                if ci == 0:
                    nc.vector.tensor_copy(v3[:, :, 64:65],
                                          ones_h[:, 0:12].unsqueeze(2))
                vps = ps.tile([P, 512], F32, tag="mm", bufs=2, name="vps")
                for d in range(6):
                    nc.tensor.matmul(vps[:, 0:cw], xT[b][d][:, P * t:P * (t + 1)],
                                     wv_h[d][:, c0:c0 + cw],
                                     start=(d == 0), stop=(d == 5))
                nc.vector.tensor_copy(
                    v3[:, (c0 // HD):((c0 + cw) // HD), 0:HD],
                    vps[:, 0:cw].rearrange("p (h c) -> p h c", c=HD))

            # b_final = W_proj^T b_v + b_proj, broadcast to [128, 768] f32
            bfin_f = sb.tile([1, D], F32, tag="bfin_f", bufs=1, name="bfin_f")
            bfin_d = dp.tile([1, D], F32, tag="bfin_d", bufs=1, name="bfin_d")

            def emit_bfinal(ci):
                c0, cw = ((0, 512), (512, 256))[ci]
                bf_ps = ps.tile([1, 512], F32, tag="mm", bufs=2, name="bf_ps")
                for d in range(6):
                    nc.tensor.matmul(bf_ps[:, 0:cw], bv_h[:, d:d + 1],
                                     wp_h[d][:, c0:c0 + cw],
                                     start=(d == 0), stop=(d == 5))
                nc.vector.tensor_add(bfin_f[:, c0:c0 + cw], bf_ps[0:1, 0:cw],
                                     bproj_row[:, c0:c0 + cw])
                if ci == 1:
                    nc.sync.dma_start(bfin_d[:], bfin_f[:])
                    nc.sync.dma_start(bfin_bc[:], bfin_d[:].to_broadcast((P, D)))

            def emit_proj_half(b, t, ci):
                c0, cw = ((0, 512), (512, 256))[ci]
                pps = ps.tile([P, 512], F32, tag="mm", bufs=2, name="pps")
                for d in range(6):
                    nc.tensor.matmul(pps[:, 0:cw],
                                     at[b][d][:, P * t:P * (t + 1)],
                                     wp_h[d][:, c0:c0 + cw],
                                     start=(d == 0), stop=(d == 5))
                osb = sb.tile([P, 512], F16, tag="outs", bufs=2, name="osb")
                nc.vector.tensor_add(osb[:, 0:cw], pps[:, 0:cw],
                                     bfin_bc[:, c0:c0 + cw])
                nc.sync.dma_start(out_il[b, t][:, c0:c0 + cw], osb[:, 0:cw])

            def emit_attention(b, fillers, pops):
                for nh in range(2):
                    n0 = 512 * nh
                    for jp in range(6):
                        qt, kt = qk[b][jp], qk[b][6 + jp]
                        ot = [ps.tile([65, 512], F32, tag="ot", bufs=2,
                                      name="otps") for _ in range(2)]
                        es = []

                        def do_av(m):
                            e = es[m]
                            for hh in range(2):
                                h = 2 * jp + hh
                                nc.tensor.matmul(
                                    ot[hh][:],
                                    v[b][m][:, 65 * h:65 * h + 65],
                                    e[:, 512 * hh:512 * (hh + 1)],
                                    start=(m == 0), stop=(m == 7))

                        # m-steps batched in pairs: the two K=64 QK pair-
                        # slots sit adjacent (one PE tile-mode switch per
                        # batch), sps double-buffers so QK(mb+1) never
                        # waits on exp(mb) reading PSUM.
                        for mb in range(4):
                            if mb > 0:
                                do_av(2 * mb - 2)
                                do_av(2 * mb - 1)
                            sps2 = []
                            for mi in range(2):
                                m = 2 * mb + mi
                                sps = ps.tile([P, N], F32, tag="s", bufs=2,
                                              name="sps")
                                for hh in range(2):
                                    r0, r1 = HD * hh, HD * (hh + 1)
                                    nc.tensor.matmul(
                                        sps[:, 512 * hh:512 * (hh + 1)],
                                        kt[r0:r1, P * m:P * (m + 1)],
                                        qt[r0:r1, n0:n0 + 512],
                                        start=True, stop=True)
                                sps2.append(sps)
                            for mi in range(2):
                                e = sb.tile([P, N], F16, tag="e", bufs=3,
                                            name="e")
                                nc.scalar.activation(e[:], sps2[mi][:], AF.Exp,
                                                     scale=SCALE)
                                es.append(e)
                            npop = pops - 3 if mb == 0 else 1
                            for _ in range(npop):
                                if fillers:
                                    fillers.pop(0)()
                        do_av(6)
                        do_av(7)
                        # normalize: evacuate O^T + denominator rows out of
                        # PSUM immediately (releases ot for the next pair;
                        # keeping the DMA broadcast round-trip inside the
                        # PSUM lifetime stalls the next pair's AV). Both
                        # heads batched: one recip/cast/mul per pair; the
                        # tiny round-trip DMAs ride the idle gpsimd queue.
                        last = b == 1 and nh == 1 and jp == 5
                        u2 = sb.tile([P, 512], F16, tag="u_sb", bufs=2,
                                     name="u2")
                        dr2 = sb.tile([1, 1024], F32, tag="dr_f", bufs=2,
                                      name="dr2")
                        for hh in range(2):
                            # last pair: denominator rows first (recip
                            # critical path), O^T evacuation on the idle
                            # ACT engine in parallel with DVE's recip
                            nc.vector.tensor_copy(
                                dr2[0:1, 512 * hh:512 * (hh + 1)],
                                ot[hh][64:65, :])
                            if last:
                                nc.scalar.copy(u2[HD * hh:HD * (hh + 1), :],
                                               ot[hh][0:HD, :])
                            else:
                                nc.vector.tensor_copy(
                                    u2[HD * hh:HD * (hh + 1), :],
                                    ot[hh][0:HD, :])
                        rr_f = sb.tile([1, 1024], F32, tag="rr_f", bufs=2,
                                       name="rr_f")
                        nc.vector.reciprocal_approx_fast(out=rr_f[:],
                                                         in_=dr2[:])
                        rr_h = sb.tile([1, 1024], F16, tag="rr_h", bufs=2,
                                       name="rr_h")
                        nc.vector.tensor_copy(rr_h[:], rr_f[:])
                        if last:
                            # final pair: PE is idle and the tail projs
                            # wait on this -- broadcast via two rank-1
                            # matmuls instead of the DMA round-trip; the
                            # keep-warm MMs cover the DVE recip latency
                            keep_warm(12)
                            bc_ps = ps.tile([P, 512], F32, tag="s", bufs=2,
                                            name="bc_ps")
                            for hh in range(2):
                                nc.tensor.matmul(
                                    bc_ps[HD * hh:HD * (hh + 1), :],
                                    ones_h[0:1, 0:HD],
                                    rr_h[0:1, 512 * hh:512 * (hh + 1)],
                                    start=True, stop=True)
                            nc.vector.tensor_mul(at[b][jp][:, n0:n0 + 512],
                                                 u2[:], bc_ps[:])
                        else:
                            rr_d = dp.tile([1, 1024], F16, tag="rr_d", bufs=2,
                                           name="rr_d")
                            nc.gpsimd.dma_start(rr_d[:], rr_h[:])
                            bc_h = sb.tile([P, 512], F16, tag="bc_h", bufs=2,
                                           name="bc_h")
                            for hh in range(2):
                                nc.gpsimd.dma_start(
                                    bc_h[HD * hh:HD * (hh + 1), :],
                                    rr_d[0:1, 512 * hh:512 * (hh + 1)]
                                    .to_broadcast((HD, 512)))
                            nc.vector.tensor_mul(at[b][jp][:, n0:n0 + 512],
                                                 u2[:], bc_h[:])

            # NOTE: pair (jp, nh) reads kt = qk[6+jp] over ALL 1024 columns
            # (keys span every token; nh only halves the query side), so a
            # pair needs BOTH nh-halves of its K tile before it starts.

            # ---------- prologue compute for attention-0 ----------
            # All V groups up front (need only the W_v columns, which
            # arrive first); the three prefix Q/K groups trail and absorb
            # the tail of the W_qk DMA stream.
            for ci in range(2):
                for t in range(8):
                    emit_v_group(0, t, ci)
            emit_qkv_group(0, 0, 0)
            emit_qkv_group(0, 6, 0)
            emit_qkv_group(0, 6, 1)

            # ---------- attention-0 fillers (48 pops = 12 pairs x 4) ------
            fill0 = []
            for jp in range(1, 6):
                fill0 += [lambda j=jp: emit_qkv_group(0, j, 0),
                          lambda j=jp + 6: emit_qkv_group(0, j, 0),
                          lambda j=jp + 6: emit_qkv_group(0, j, 1)]
            fill0 += [lambda j=jp: emit_qkv_group(0, j, 1) for jp in range(6)]
            fill0 += [lambda ci=ci: emit_bfinal(ci) for ci in range(2)]
            fill0 += [lambda tb=tb, j=j: emit_transpose(1, tb, j)
                      for tb in range(2) for j in range(6)]
            fill0 += [lambda t=t, ci=ci: emit_v_group(1, t, ci)
                      for ci in range(2) for t in range(8)][:13]

            emit_attention(0, fill0, pops=4)
            for f in fill0:
                f()

            # ---------- between attentions: b1 pair-0 Q/K ----------
            emit_qkv_group(1, 0, 0)
            emit_qkv_group(1, 6, 0)
            emit_qkv_group(1, 6, 1)

            # ---------- attention-1 fillers (48 pops = 12 pairs x 4) ------
            fill1 = []
            for jp in range(1, 6):
                fill1 += [lambda j=jp: emit_qkv_group(1, j, 0),
                          lambda j=jp + 6: emit_qkv_group(1, j, 0),
                          lambda j=jp + 6: emit_qkv_group(1, j, 1)]
            fill1 += [lambda t=t: emit_v_group(1, t, 1) for t in range(5, 8)]
            fill1 += [lambda j=jp: emit_qkv_group(1, j, 1) for jp in range(6)]
            fill1 += [lambda t=t, ci=ci: emit_proj_half(0, t, ci)
                      for t in range(8) for ci in range(2)]
            fill1 += [lambda t=t, ci=ci: emit_proj_half(1, t, ci)
                      for t in range(4) for ci in range(2)]

            emit_attention(1, fill1, pops=4)
            for f in fill1:
                f()

            # ---------- tail: b1 proj second half ----------
            for t in range(4, 8):
                for ci in range(2):
                    emit_proj_half(1, t, ci)
    nc.compile()
    return nc


def _get_nc():
    if "nc" not in _CACHE:
        _CACHE["nc"] = _build()
    return _CACHE["nc"]


def _in_maps(x, W_qkv, b_qkv, W_proj, b_proj):
    x = np.ascontiguousarray(x, dtype=np.float16)
    return [
        {
            "x": x[2 * i:2 * i + 2],
            "W_qkv": np.ascontiguousarray(W_qkv, dtype=np.float16),
            "b_qkv": np.ascontiguousarray(b_qkv, dtype=np.float32),
            "W_proj": np.ascontiguousarray(W_proj, dtype=np.float16),
            "b_proj": np.ascontiguousarray(b_proj, dtype=np.float32),
        }
        for i in range(8)
    ]


def kernel(x, W_qkv, b_qkv, W_proj, b_proj):
    from concourse.bass_utils import run_bass_kernel_spmd

    nc = _get_nc()
    in_maps = _in_maps(x, W_qkv, b_qkv, W_proj, b_proj)
    res = run_bass_kernel_spmd(nc, in_maps, core_ids=list(range(8)))
    return np.concatenate(
        [r["out"].astype(np.float32) for r in res.results], axis=0)


# revision 50
# speedup vs baseline: 1.0163x; 1.0163x over previous
"""Multi-head self-attention (B=16, N=1024, D=768, H=12) on 8 TRN2 NeuronCores.

Data-parallel over batch (2 batches per core, weights replicated, no
collectives). Per core, one fused Bass/Tile kernel. ~354us HW exec
(v1 baseline: 453.8us), rel err 3.7e-4.

Layout: token 8p+t lives at partition p, slot t (attention is
permutation-invariant over tokens; undone at the out DMA). QT/KT in
[dim, tok] f16; V_aug = [x W_v | ones-col per head]; per head
S^T = K Q^T (row-tiled pairs: the two K=64 matmuls of a head pair run
CONCURRENTLY in the top/bottom PE array halves, tile_position
auto-derived from base_partition), E = exp(S^T*scale) on ACT,
O^T = V_aug^T E with row 64 = softmax denominator (ones column is free:
matmul cost is N-streaming, M=65 vs 64 costs nothing). All matmul
operands f16 (fp32 runs 1/4 rate and breaks HAM warm-up), PSUM f32.

What this version does beyond the v1 structure (each item measured):
  - host-side f16 casts for x/W_qkv/W_proj, f16 output (halves DMA
    bytes; on-chip weight-cast DVE work removed)
  - DMA issue engines chosen deliberately: each dma_start costs ~0.6us
    of the ISSUING engine's time, so weights issue from the otherwise-
    idle gpsimd queue (ACT must stay free for transpose evacuation),
    x6 + qkv-bias ride sync; softmax-recip round-trip DMAs ride gpsimd
  - W_v columns land first, in their own tiles, so all 16 V groups run
    while the 2x larger Q/K columns are still streaming
  - x transposes as REGULAR matmuls (chunk^T @ I): ~56ns pipelined vs
    ~250ns transpose-mode, and they count as HAM activity; 4 share one
    PSUM bank -> single [128,512] evacuation copy (b0 on ACT, b1 DVE)
  - rank-1 bias matmuls removed: fused bias (W_proj^T b_v + b_proj)
    broadcast once to [128,768] f32, added by DVE during PSUM evacuation
  - attention m-steps batched in pairs: the two K=64 QK pair-slots sit
    adjacent (one PE tile-mode switch in/out per batch, ~+100ns each),
    sps double-buffered so QK(mb+1) never waits exp(mb)'s PSUM read
  - nh-outer pair order; fillers scheduled against pop deadlines:
    attention-0 hosts b0 Q/K groups + b1 transposes + b1 V, attention-1
    hosts b1 Q/K + b0 proj + half of b1 proj; pair (jp, nh) needs BOTH
    nh-halves of its K tile (keys span all tokens) before it starts
  - O^T + denominator evacuated from PSUM immediately (anything slower
    -- e.g. DMA round-trips inside the ot lifetime -- stalls the next
    pair's AV by 2-4us); recip must NOT read PSUM directly (HW gives
    wrong values; sim diverges); both heads share one recip/cast/mul
  - final pair's softmax broadcast via two rank-1 PE matmuls (PE is
    idle there) + keep-warm matmuls over the recip latency so the tail
    projections run at 8/8 clock
  - qk tiles share slots across batches (qk[1] first written only after
    attention-0 drains); at tiles are per-batch (sharing would deadlock
    the DVE FIFO against PE via the b0-proj fillers)

Engine budget at 354us: PE ~91% busy (the bottleneck), ACT ~60% (192
exps at ~1.1us), DVE ~55%, HBM ~8MB in / 3MB out per core.
"""

import numpy as np

_CACHE: dict = {}

P = 128
BL, N, D, H, HD = 2, 1024, 768, 12, 64
D3 = 3 * D
SCALE = float(HD) ** -0.5


def _build():
    import concourse.mybir as mybir
    import concourse.tile as tile
    from concourse import bacc
    from concourse.masks import make_identity

    dt = mybir.dt
    F32, F16 = dt.float32, dt.float16
    AF = mybir.ActivationFunctionType

    nc = bacc.Bacc("TRN2", target_bir_lowering=False, debug=False)
    x_d = nc.dram_tensor("x", [BL, N, D], F16, kind="ExternalInput").ap()
    wqkv_d = nc.dram_tensor("W_qkv", [D, D3], F16, kind="ExternalInput").ap()
    bqkv_d = nc.dram_tensor("b_qkv", [D3], F32, kind="ExternalInput").ap()
    wproj_d = nc.dram_tensor("W_proj", [D, D], F16, kind="ExternalInput").ap()
    bproj_d = nc.dram_tensor("b_proj", [D], F32, kind="ExternalInput").ap()
    out_d = nc.dram_tensor("out", [BL, N, D], F16, kind="ExternalOutput").ap()
    # token-interleaved views: partition p, slot t <-> token 8p+t
    x_il = x_d.rearrange("b (p i) d -> b p (i d)", p=P)       # [2, 128, 6144]
    out_il = out_d.rearrange("b (p i) d -> b i p d", p=P)     # [2, 8, 128, 768]

    with tile.TileContext(nc) as tc:
        with tc.tile_pool(name="sb", bufs=1) as sb, \
             tc.tile_pool(name="dp", bufs=1, space="DRAM") as dp, \
             tc.tile_pool(name="ps", bufs=2, space="PSUM") as ps:

            # ---------- constants ----------
            ident = sb.tile([P, P], F16, tag="ident", bufs=1, name="ident")
            make_identity(nc, ident[:])
            ones_h = sb.tile([P, P], F16, tag="ones_h", bufs=1, name="ones_h")
            nc.vector.memset(ones_h[:], 1.0)

            # ---------- input DMAs ----------
            # sync queue: ONLY x6[0] (fast path for the first transposes).
            # scalar queue: biases, W_qkv (in d order, consumed
            # incrementally by the first QKV groups), then x6[1]
            # (needed only mid-attention-0), then W_proj.
            # DMA issue engines matter: each dma_start costs ~0.6us of the
            # ISSUING engine's time. ACT (scalar) must stay free for the
            # transpose-evacuation copies, so weights issue from the idle
            # gpsimd queue; x6 + the tiny qkv-bias ride the sync queue.
            bstg = sb.tile([18, P], F32, tag="bstg", bufs=1, name="bstg")
            nc.sync.dma_start(bstg[:], bqkv_d.rearrange("(j p) -> j p", p=P))
            x6 = {}
            for b in range(BL):
                x6[b] = sb.tile([P, 8 * D], F16, tag="x6", bufs=2, name="x6")
                for q in range(4):
                    nc.sync.dma_start(x6[b][:, 2 * D * q:2 * D * (q + 1)],
                                      x_il[b][:, 2 * D * q:2 * D * (q + 1)])
            # W_v columns land FIRST (own tiles -- a shared tile would
            # serialize V groups behind the whole-tile DMA set) so all 16
            # V groups can run while the 2x larger Q/K columns stream in.
            wq_h, wv_h, wp_h = [], [], []
            for d in range(6):
                t = sb.tile([P, D], F16, tag=f"wv{d}", bufs=1, name=f"wv{d}")
                nc.gpsimd.dma_start(t[:], wqkv_d[P * d:P * (d + 1), 2 * D:D3])
                wv_h.append(t)
            for d in range(6):
                t = sb.tile([P, 2 * D], F16, tag=f"wqkv{d}", bufs=1,
                            name=f"wqkv{d}")
                nc.gpsimd.dma_start(t[:], wqkv_d[P * d:P * (d + 1), 0:2 * D])
                wq_h.append(t)
            bproj_row = sb.tile([1, D], F32, tag="bproj_row", bufs=1,
                                name="bproj_row")
            nc.gpsimd.dma_start(bproj_row[:], bproj_d.unsqueeze(0))
            for d in range(6):
                t = sb.tile([P, D], F16, tag=f"wproj{d}", bufs=1, name=f"wproj{d}")
                nc.gpsimd.dma_start(t[:], wproj_d[P * d:P * (d + 1), :])
                wp_h.append(t)

            warm_h = sb.tile([P, 512], F16, tag="warm", bufs=1, name="warm_h")
            nc.vector.memset(warm_h[:], 0.0)

            def keep_warm(n):
                # ~3.4us of dense matmuls flips HAM to 8/8 (16 x 215ns)
                for wi in range(n):
                    wps = ps.tile([P, 512], F32, tag="mm", bufs=2, name="wps")
                    nc.tensor.matmul(wps[:], ones_h[:, 0:P], warm_h[:],
                                     start=True, stop=True)

            # PE warm-up: bridges the DGE-ramp / x6-DMA wait and flips HAM
            keep_warm(18)

            # ---------- x transposes as regular matmuls (chunk^T @ I) ----
            # 4 transposes share one PSUM bank; one [128,512] copy out.
            # b0 copies on ACT (idle in prologue), b1 on DVE.
            xT = {b: [sb.tile([P, N], F16, tag=f"xT{b}_{j}", bufs=1,
                              name=f"xT{b}_{j}") for j in range(6)]
                  for b in range(BL)}

            def emit_transpose(b, tb, j):
                bank = ps.tile([P, 512], F32, tag="mm", bufs=2, name="tbank")
                for k in range(4):
                    t = 4 * tb + k
                    nc.tensor.matmul(
                        bank[:, P * k:P * (k + 1)],
                        x6[b][:, D * t + P * j:D * t + P * (j + 1)],
                        ident[:], start=True, stop=True)
                if b == 0:
                    nc.scalar.copy(xT[b][j][:, 512 * tb:512 * (tb + 1)],
                                   bank[:])
                else:
                    nc.vector.tensor_copy(
                        xT[b][j][:, 512 * tb:512 * (tb + 1)], bank[:])

            for tb in range(2):
                for j in range(6):
                    emit_transpose(0, tb, j)

            # ---------- qkv bias transpose (tiny) ----------
            bstg_h = sb.tile([18, P], F16, tag="bstg_h", bufs=1, name="bstg_h")
            nc.vector.tensor_copy(bstg_h[:], bstg[:])
            btp = ps.tile([P, 18], F16, tag="mm", bufs=2, name="btp")
            nc.tensor.transpose(btp[:], bstg_h[:], ident[0:18, 0:18])
            bqkvT = sb.tile([P, 18], F32, tag="bqkvT", bufs=1, name="bqkvT")
            nc.vector.tensor_copy(bqkvT[:], btp[:])
            bv_h = sb.tile([P, 6], F16, tag="bv_h", bufs=1, name="bv_h")
            nc.vector.tensor_copy(bv_h[:], btp[:, 12:18])

            # ---------- tiles ----------
            # qk tags shared across batches (qk[1] first written only after
            # attention-0 fully drains -> clean slot WAR). at tags are
            # per-batch: attention-1's normalization writes at[1] while
            # b0-proj fillers still read at[0] -> sharing would deadlock
            # the DVE FIFO against the PE FIFO.
            qk = {b: [sb.tile([P, N], F16, tag=f"qk_{j}", bufs=1,
                              name=f"qk{j}") for j in range(12)]
                  for b in range(BL)}
            v = {b: [sb.tile([P, 12 * 65], F16, tag=f"v{b % 2}_{t}", bufs=1,
                             name=f"v{t}") for t in range(8)]
                 for b in range(BL)}
            at = {b: [sb.tile([P, N], F16, tag=f"at{b}_{j}", bufs=1,
                              name=f"at{j}") for j in range(6)]
                  for b in range(BL)}
            bfin_bc = sb.tile([P, D], F32, tag="bfin_bc", bufs=1,
                              name="bfin_bc")

            def emit_qkv_group(b, j, nh):
                qps = ps.tile([P, 512], F32, tag="mm", bufs=2, name="qps")
                for d in range(6):
                    nc.tensor.matmul(qps[:], wq_h[d][:, P * j:P * (j + 1)],
                                     xT[b][d][:, 512 * nh:512 * (nh + 1)],
                                     start=(d == 0), stop=(d == 5))
                nc.vector.tensor_scalar_add(
                    qk[b][j][:, 512 * nh:512 * (nh + 1)], qps[:],
                    bqkvT[:, j:j + 1])

            def emit_v_group(b, t, ci):
                c0, cw = ((0, 512), (512, 256))[ci]
                v3 = v[b][t].rearrange("p (h c) -> p h c", c=65)
                if ci == 0:
                    nc.vector.tensor_copy(v3[:, :, 64:65],
                                          ones_h[:, 0:12].unsqueeze(2))
                vps = ps.tile([P, 512], F32, tag="mm", bufs=2, name="vps")
                for d in range(6):
                    nc.tensor.matmul(vps[:, 0:cw], xT[b][d][:, P * t:P * (t + 1)],
                                     wv_h[d][:, c0:c0 + cw],
                                     start=(d == 0), stop=(d == 5))
                nc.vector.tensor_copy(
                    v3[:, (c0 // HD):((c0 + cw) // HD), 0:HD],
                    vps[:, 0:cw].rearrange("p (h c) -> p h c", c=HD))

            # b_final = W_proj^T b_v + b_proj, broadcast to [128, 768] f32
            bfin_f = sb.tile([1, D], F32, tag="bfin_f", bufs=1, name="bfin_f")
            bfin_d = dp.tile([1, D], F32, tag="bfin_d", bufs=1, name="bfin_d")

            def emit_bfinal(ci):
                c0, cw = ((0, 512), (512, 256))[ci]
                bf_ps = ps.tile([1, 512], F32, tag="mm", bufs=2, name="bf_ps")
                for d in range(6):
                    nc.tensor.matmul(bf_ps[:, 0:cw], bv_h[:, d:d + 1],
                                     wp_h[d][:, c0:c0 + cw],
                                     start=(d == 0), stop=(d == 5))
                nc.vector.tensor_add(bfin_f[:, c0:c0 + cw], bf_ps[0:1, 0:cw],
                                     bproj_row[:, c0:c0 + cw])
                if ci == 1:
                    nc.sync.dma_start(bfin_d[:], bfin_f[:])
                    nc.sync.dma_start(bfin_bc[:], bfin_d[:].to_broadcast((P, D)))

            def emit_proj_half(b, t, ci):
                c0, cw = ((0, 512), (512, 256))[ci]
                pps = ps.tile([P, 512], F32, tag="mm", bufs=2, name="pps")
                for d in range(6):
                    nc.tensor.matmul(pps[:, 0:cw],
                                     at[b][d][:, P * t:P * (t + 1)],
                                     wp_h[d][:, c0:c0 + cw],
                                     start=(d == 0), stop=(d == 5))
                osb = sb.tile([P, 512], F16, tag="outs", bufs=2, name="osb")
                nc.vector.tensor_add(osb[:, 0:cw], pps[:, 0:cw],
                                     bfin_bc[:, c0:c0 + cw])
                nc.sync.dma_start(out_il[b, t][:, c0:c0 + cw], osb[:, 0:cw])

            def emit_attention(b, fillers, pops):
                deferred_mul = []
                for nh in range(2):
                    n0 = 512 * nh
                    for jp in range(6):
                        qt, kt = qk[b][jp], qk[b][6 + jp]
                        ot = [ps.tile([65, 512], F32, tag="ot", bufs=2,
                                      name="otps") for _ in range(2)]
                        es = []

                        def do_av(m):
                            e = es[m]
                            for hh in range(2):
                                h = 2 * jp + hh
                                nc.tensor.matmul(
                                    ot[hh][:],
                                    v[b][m][:, 65 * h:65 * h + 65],
                                    e[:, 512 * hh:512 * (hh + 1)],
                                    start=(m == 0), stop=(m == 7))

                        # m-steps batched in pairs: the two K=64 QK pair-
                        # slots sit adjacent (one PE tile-mode switch per
                        # batch), sps double-buffers so QK(mb+1) never
                        # waits on exp(mb) reading PSUM.
                        for mb in range(4):
                            if mb > 0:
                                do_av(2 * mb - 2)
                                do_av(2 * mb - 1)
                            sps2 = []
                            for mi in range(2):
                                m = 2 * mb + mi
                                sps = ps.tile([P, N], F32, tag="s", bufs=2,
                                              name="sps")
                                for hh in range(2):
                                    r0, r1 = HD * hh, HD * (hh + 1)
                                    nc.tensor.matmul(
                                        sps[:, 512 * hh:512 * (hh + 1)],
                                        kt[r0:r1, P * m:P * (m + 1)],
                                        qt[r0:r1, n0:n0 + 512],
                                        start=True, stop=True)
                                sps2.append(sps)
                            for mi in range(2):
                                e = sb.tile([P, N], F16, tag="e", bufs=3,
                                            name="e")
                                nc.scalar.activation(e[:], sps2[mi][:], AF.Exp,
                                                     scale=SCALE)
                                es.append(e)
                            npop = pops - 3 if mb == 0 else 1
                            for _ in range(npop):
                                if fillers:
                                    fillers.pop(0)()
                        do_av(6)
                        do_av(7)
                        # normalize: evacuate O^T + denominator rows out of
                        # PSUM immediately (releases ot for the next pair;
                        # keeping the DMA broadcast round-trip inside the
                        # PSUM lifetime stalls the next pair's AV). Both
                        # heads batched: one recip/cast/mul per pair; the
                        # tiny round-trip DMAs ride the idle gpsimd queue.
                        last = b == 1 and nh == 1 and jp == 5
                        u2 = sb.tile([P, 512], F16, tag="u_sb", bufs=2,
                                     name="u2")
                        dr2 = sb.tile([1, 1024], F32, tag="dr_f", bufs=2,
                                      name="dr2")
                        for hh in range(2):
                            # last pair: denominator rows first (recip
                            # critical path), O^T evacuation on the idle
                            # ACT engine in parallel with DVE's recip
                            nc.vector.tensor_copy(
                                dr2[0:1, 512 * hh:512 * (hh + 1)],
                                ot[hh][64:65, :])
                            if last:
                                nc.scalar.copy(u2[HD * hh:HD * (hh + 1), :],
                                               ot[hh][0:HD, :])
                            else:
                                nc.vector.tensor_copy(
                                    u2[HD * hh:HD * (hh + 1), :],
                                    ot[hh][0:HD, :])
                        rr_f = sb.tile([1, 1024], F32, tag="rr_f", bufs=2,
                                       name="rr_f")
                        nc.vector.reciprocal_approx_fast(out=rr_f[:],
                                                         in_=dr2[:])
                        rr_h = sb.tile([1, 1024], F16, tag="rr_h", bufs=2,
                                       name="rr_h")
                        nc.vector.tensor_copy(rr_h[:], rr_f[:])
                        if last:
                            # final pair: PE is idle and the tail projs
                            # wait on this -- broadcast via two rank-1
                            # matmuls instead of the DMA round-trip; the
                            # keep-warm MMs cover the DVE recip latency
                            keep_warm(12)
                            bc_ps = ps.tile([P, 512], F32, tag="s", bufs=2,
                                            name="bc_ps")
                            for hh in range(2):
                                nc.tensor.matmul(
                                    bc_ps[HD * hh:HD * (hh + 1), :],
                                    ones_h[0:1, 0:HD],
                                    rr_h[0:1, 512 * hh:512 * (hh + 1)],
                                    start=True, stop=True)
                            # second-to-last pair's deferred mul first:
                            # its broadcast DMA landed long ago, so the
                            # DVE queue flows straight into our mul
                            for args in deferred_mul:
                                nc.vector.tensor_mul(*args)
                            deferred_mul = []
                            nc.vector.tensor_mul(at[b][jp][:, n0:n0 + 512],
                                                 u2[:], bc_ps[:])
                        else:
                            rr_d = dp.tile([1, 1024], F16, tag="rr_d", bufs=2,
                                           name="rr_d")
                            nc.gpsimd.dma_start(rr_d[:], rr_h[:])
                            bc_h = sb.tile([P, 512], F16, tag="bc_h", bufs=2,
                                           name="bc_h")
                            for hh in range(2):
                                nc.gpsimd.dma_start(
                                    bc_h[HD * hh:HD * (hh + 1), :],
                                    rr_d[0:1, 512 * hh:512 * (hh + 1)]
                                    .to_broadcast((HD, 512)))
                            if b == 1 and nh == 1 and jp == 4:
                                # defer: emitting this mul now would park
                                # the DVE FIFO on the DMA round-trip and
                                # delay the final pair's recip chain
                                deferred_mul.append(
                                    (at[b][jp][:, n0:n0 + 512], u2[:],
                                     bc_h[:]))
                            else:
                                nc.vector.tensor_mul(
                                    at[b][jp][:, n0:n0 + 512],
                                    u2[:], bc_h[:])

            # NOTE: pair (jp, nh) reads kt = qk[6+jp] over ALL 1024 columns
            # (keys span every token; nh only halves the query side), so a
            # pair needs BOTH nh-halves of its K tile before it starts.

            # ---------- prologue compute for attention-0 ----------
            # All V groups up front (need only the W_v columns, which
            # arrive first); the three prefix Q/K groups trail and absorb
            # the tail of the W_qk DMA stream.
            for ci in range(2):
                for t in range(8):
                    emit_v_group(0, t, ci)
            emit_qkv_group(0, 0, 0)
            emit_qkv_group(0, 6, 0)
            emit_qkv_group(0, 6, 1)

            # ---------- attention-0 fillers (48 pops = 12 pairs x 4) ------
            fill0 = []
            for jp in range(1, 6):
                fill0 += [lambda j=jp: emit_qkv_group(0, j, 0),
                          lambda j=jp + 6: emit_qkv_group(0, j, 0),
                          lambda j=jp + 6: emit_qkv_group(0, j, 1)]
            fill0 += [lambda j=jp: emit_qkv_group(0, j, 1) for jp in range(6)]
            fill0 += [lambda ci=ci: emit_bfinal(ci) for ci in range(2)]
            fill0 += [lambda tb=tb, j=j: emit_transpose(1, tb, j)
                      for tb in range(2) for j in range(6)]
            fill0 += [lambda t=t, ci=ci: emit_v_group(1, t, ci)
                      for ci in range(2) for t in range(8)][:13]

            emit_attention(0, fill0, pops=4)
            for f in fill0:
                f()

            # ---------- between attentions: b1 pair-0 Q/K ----------
            emit_qkv_group(1, 0, 0)
            emit_qkv_group(1, 6, 0)
            emit_qkv_group(1, 6, 1)

            # ---------- attention-1 fillers (48 pops = 12 pairs x 4) ------
            fill1 = []
            for jp in range(1, 6):
                fill1 += [lambda j=jp: emit_qkv_group(1, j, 0),
                          lambda j=jp + 6: emit_qkv_group(1, j, 0),
                          lambda j=jp + 6: emit_qkv_group(1, j, 1)]
            fill1 += [lambda t=t: emit_v_group(1, t, 1) for t in range(5, 8)]
            fill1 += [lambda j=jp: emit_qkv_group(1, j, 1) for jp in range(6)]
            fill1 += [lambda t=t, ci=ci: emit_proj_half(0, t, ci)
                      for t in range(8) for ci in range(2)]
            fill1 += [lambda t=t, ci=ci: emit_proj_half(1, t, ci)
                      for t in range(4) for ci in range(2)]

            emit_attention(1, fill1, pops=4)
            for f in fill1:
                f()

            # ---------- tail: b1 proj second half ----------
            for t in range(4, 8):
                for ci in range(2):
                    emit_proj_half(1, t, ci)
    nc.compile()
    return nc


def _get_nc():
    if "nc" not in _CACHE:
        _CACHE["nc"] = _build()
    return _CACHE["nc"]


def _in_maps(x, W_qkv, b_qkv, W_proj, b_proj):
    x = np.ascontiguousarray(x, dtype=np.float16)
    return [
        {
            "x": x[2 * i:2 * i + 2],
            "W_qkv": np.ascontiguousarray(W_qkv, dtype=np.float16),
            "b_qkv": np.ascontiguousarray(b_qkv, dtype=np.float32),
            "W_proj": np.ascontiguousarray(W_proj, dtype=np.float16),
            "b_proj": np.ascontiguousarray(b_proj, dtype=np.float32),
        }
        for i in range(8)
    ]


def kernel(x, W_qkv, b_qkv, W_proj, b_proj):
    from concourse.bass_utils import run_bass_kernel_spmd

    nc = _get_nc()
    in_maps = _in_maps(x, W_qkv, b_qkv, W_proj, b_proj)
    res = run_bass_kernel_spmd(nc, in_maps, core_ids=list(range(8)))
    return np.concatenate(
        [r["out"].astype(np.float32) for r in res.results], axis=0)
